# revision 12
# baseline (speedup 1.0000x reference)
"""Bi-LSTM-CRF loss kernel for Trainium2 (8 NeuronCores, data-parallel over batch).

Per-core layout (B_c = 8 sequences):
  - embedding gather via indirect DMA, PE-transpose to xT [E=128, tok]
  - 2 bidirectional LSTM layers; recurrence keeps batch on PSUM partitions
    (f-dir rows 0:8, b-dir rows 32:40 for tensor-engine col-group concurrency),
    gates [i,f,o,g]-reordered so one sigmoid covers i,f,o.
  - layer-1 Wx bulk-precomputed to HBM, streamed back, added via selector-matmul
  - CRF numerator via one-hot reductions + matmul partition-reduce
  - CRF normalizer: sequential 512-step logsumexp scan on 8 partitions
Host side only does sharding / layout / dtype prep and the final gather.
"""
import sys
sys.path.insert(0, '/opt/trn_rl_repo')
import numpy as np

NCORES = 8
B, L, E, H, V, T = 64, 512, 128, 256, 30552, 9
G = 4 * H          # 1024
Bc = B // NCORES   # 8
# gate reorder [i, f, o, g]
PERM = np.r_[0:256, 256:512, 768:1024, 512:768]

_CACHE = {}


def _build(Lb):
    import concourse.bass as bass
    import concourse.bacc as bacc
    import concourse.mybir as mybir
    import concourse.tile as tile

    f32 = mybir.dt.float32
    i32 = mybir.dt.int32
    AF = mybir.ActivationFunctionType
    OP = mybir.AluOpType
    AX = mybir.AxisListType

    NT = Lb * Bc            # tokens per core
    NC2 = NT // 128         # gather chunks
    CC = NT // 128          # label-grid free dim (Lb*Bc/128)

    nc = bacc.Bacc("TRN2", target_bir_lowering=False, debug=False)

    # ---- inputs ----
    emb_d = nc.dram_tensor("emb", (V, E), f32, kind="ExternalInput")
    ident_d = nc.dram_tensor("ident", (128, 128), f32, kind="ExternalInput")
    ones_d = nc.dram_tensor("ones", (1, 128), f32, kind="ExternalInput")
    idx2_d = nc.dram_tensor("idx2", (128, NC2), i32, kind="ExternalInput")
    wih0_d = {d: nc.dram_tensor(f"wih0{d}", (E, G), f32, kind="ExternalInput") for d in "fb"}
    whh0_d = {d: nc.dram_tensor(f"whh0{d}", (H, G), f32, kind="ExternalInput") for d in "fb"}
    b0_d = {d: nc.dram_tensor(f"b0{d}", (1, G), f32, kind="ExternalInput") for d in "fb"}
    wih1_d = {d: nc.dram_tensor(f"wih1{d}", (2 * H, G), f32, kind="ExternalInput") for d in "fb"}
    whh1_d = {d: nc.dram_tensor(f"whh1{d}", (H, G), f32, kind="ExternalInput") for d in "fb"}
    b1_d = {d: nc.dram_tensor(f"b1{d}", (1, G), f32, kind="ExternalInput") for d in "fb"}
    cls_d = nc.dram_tensor("clsT", (2 * H, T), f32, kind="ExternalInput")
    clsb_d = nc.dram_tensor("clsb", (1, T), f32, kind="ExternalInput")
    start_d = nc.dram_tensor("startv", (1, T), f32, kind="ExternalInput")
    end_d = nc.dram_tensor("endv", (1, T), f32, kind="ExternalInput")
    trm_d = nc.dram_tensor("trans_rm", (1, T * T), f32, kind="ExternalInput")
    trj_d = nc.dram_tensor("trans_jm", (1, T * T), f32, kind="ExternalInput")
    lab_d = nc.dram_tensor("labs", (128, CC), i32, kind="ExternalInput")
    labp_d = nc.dram_tensor("labsp", (128, CC), i32, kind="ExternalInput")
    mask_d = nc.dram_tensor("maskf", (128, CC), f32, kind="ExternalInput")
    masknx_d = nc.dram_tensor("masknx", (128, CC), f32, kind="ExternalInput")
    bsel_d = nc.dram_tensor("bsel", (128, 8), f32, kind="ExternalInput")
    # ---- outputs ----
    preds_d = nc.dram_tensor("preds_raw", (128, CC * T), f32, kind="ExternalOutput")
    llh_d = nc.dram_tensor("llh", (Bc, 1), f32, kind="ExternalOutput")

    with tile.TileContext(nc) as tc:
        with tc.tile_pool(name="persist", bufs=1) as pp, \
             tc.tile_pool(name="dram", bufs=1, space="DRAM") as dp:
            ident = pp.tile([128, 128], f32, name="ident", tag="ident")
            nc.sync.dma_start(ident[:], ident_d[:])
            ones = pp.tile([1, 128], f32, name="ones", tag="ones")
            nc.sync.dma_start(ones[:], ones_d[:])
            idx2 = pp.tile([128, NC2], i32, name="idx2", tag="idx2")
            nc.sync.dma_start(idx2[:], idx2_d[:])

            wih0 = {}; whh0 = {}; b0 = {}; whh1 = {}; b1 = {}
            for d in "fb":
                wih0[d] = pp.tile([128, G], f32, name=f"wih0{d}", tag=f"wih0{d}")
                nc.sync.dma_start(wih0[d][:], wih0_d[d][:])
                whh0[d] = [pp.tile([128, G], f32, name=f"whh0{d}{k}", tag=f"whh0{d}{k}") for k in range(2)]
                for k in range(2):
                    nc.sync.dma_start(whh0[d][k][:], whh0_d[d][k * 128:(k + 1) * 128, :])
                b0[d] = pp.tile([1, G], f32, name=f"b0{d}", tag=f"b0{d}")
                nc.sync.dma_start(b0[d][:], b0_d[d][:])
                whh1[d] = [pp.tile([128, G], f32, name=f"whh1{d}{k}", tag=f"whh1{d}{k}") for k in range(2)]
                for k in range(2):
                    nc.sync.dma_start(whh1[d][k][:], whh1_d[d][k * 128:(k + 1) * 128, :])
                b1[d] = pp.tile([1, G], f32, name=f"b1{d}", tag=f"b1{d}")
                nc.sync.dma_start(b1[d][:], b1_d[d][:])

            X1dram = {d: dp.tile([NT, G], f32, name=f"X1{d}", tag=f"X1{d}") for d in "fb"}

            # ---------------- gather + transpose x2T ----------------
            ph0_cm = tc.tile_pool(name="ph0", bufs=1)
            ph0 = ph0_cm.__enter__()
            h0T = [ph0.tile([128, NT], f32, name=f"h0T{i}", tag=f"h0T{i}") for i in range(4)]
            x2T = ph0.tile([128, NT], f32, name="x2T", tag="x2T")
            with tc.tile_pool(name="gath", bufs=4) as gp, \
                 tc.tile_pool(name="gps", bufs=2, space="PSUM") as gps:
                for c in range(NC2):
                    gx = gp.tile([128, 128], f32, name="gx", tag="gx")
                    nc.gpsimd.indirect_dma_start(
                        out=gx[:], out_offset=None, in_=emb_d[:],
                        in_offset=bass.IndirectOffsetOnAxis(ap=idx2[:, c:c + 1], axis=0))
                    tps = gps.tile([128, 128], f32, name="gt", tag="gt", space="PSUM")
                    nc.tensor.transpose(out=tps[:], in_=gx[:], identity=ident[:])
                    nc.scalar.copy(out=x2T[:, c * 128:(c + 1) * 128], in_=tps[:])

            # ---------------- LSTM layers ----------------
            def lstm_layer(layer, hT):
                """layer 0: x-term direct from x2T; layer 1: from X1 chunks."""
                with tc.tile_pool(name=f"lz{layer}", bufs=1, space="PSUM") as zp, \
                     tc.tile_pool(name=f"lt{layer}", bufs=2, space="PSUM") as tpp, \
                     tc.tile_pool(name=f"ls{layer}", bufs=2) as sp, \
                     tc.tile_pool(name=f"lx{layer}", bufs=3) as xp:
                    zf = zp.tile([8, G], f32, name="zf", tag="zf", space="PSUM")
                    zb = zp.tile([40, G], f32, name="zb", tag="zb", space="PSUM")
                    c_t = {d: pp.tile([8, H], f32, name=f"c{layer}{d}", tag=f"c{layer}{d}") for d in "fb"}
                    for d in "fb":
                        nc.gpsimd.memset(c_t[d][:], 0.0)
                    whh = whh0 if layer == 0 else whh1
                    xch = {}
                    for t in range(Lb):
                        zz = {"f": zf[0:8, :], "b": zb[32:40, :]}
                        tok = {"f": t, "b": Lb - 1 - t}
                        if layer == 1 and t % 8 == 0:
                            xch = {d: xp.tile([64, G], f32, name=f"xc{d}", tag=f"xc{d}") for d in "fb"}
                            r0f = 8 * t
                            r0b = 8 * (Lb - 8 - t)
                            nc.sync.dma_start(xch["f"][:], X1dram["f"][r0f:r0f + 64, :])
                            nc.sync.dma_start(xch["b"][:], X1dram["b"][r0b:r0b + 64, :])
                        for hf in range(2):
                            colsl = slice(hf * 512, (hf + 1) * 512)
                            for d in "fb":
                                first = True
                                z = zz[d]
                                if layer == 0:
                                    cx = tok[d] * 8
                                    nc.tensor.matmul(z[:, colsl], lhsT=x2T[:, cx:cx + 8],
                                                     rhs=wih0[d][:, colsl], start=True, stop=False)
                                    nc.tensor.matmul(z[:, colsl], lhsT=ones[:, 0:8],
                                                     rhs=b0[d][:, colsl], start=False, stop=(t == 0))
                                    first = False
                                else:
                                    i8 = t % 8 if d == "f" else 7 - (t % 8)
                                    nc.tensor.matmul(z[:, colsl], lhsT=ident[0:64, 8 * i8:8 * i8 + 8],
                                                     rhs=xch[d][:, colsl],
                                                     start=True, stop=(t == 0))
                                    first = False
                                if t > 0:
                                    pc = {"f": (t - 1) * 8, "b": (tok["b"] + 1) * 8}[d]
                                    base = 0 if d == "f" else 2
                                    for k in range(2):
                                        nc.tensor.matmul(z[:, colsl], lhsT=hT[base + k][:, pc:pc + 8],
                                                         rhs=whh[d][k][:, colsl],
                                                         start=False, stop=(k == 1))
                        # gates
                        for d in "fb":
                            z = zz[d]
                            sig = sp.tile([8, 768], f32, name=f"sig{d}", tag=f"sig{d}")
                            tg = sp.tile([8, H], f32, name=f"tg{d}", tag=f"tg{d}")
                            nc.scalar.activation(out=sig[:], in_=z[:, 0:768], func=AF.Sigmoid)
                            nc.scalar.activation(out=tg[:], in_=z[:, 768:1024], func=AF.Tanh)
                            t1 = sp.tile([8, H], f32, name=f"t1{d}", tag=f"t1{d}")
                            nc.vector.tensor_tensor(out=t1[:], in0=sig[:, 0:256], in1=tg[:], op=OP.mult)
                            if t > 0:
                                t2 = sp.tile([8, H], f32, name=f"t2{d}", tag=f"t2{d}")
                                nc.gpsimd.tensor_tensor(out=t2[:], in0=sig[:, 256:512], in1=c_t[d][:], op=OP.mult)
                                nc.vector.tensor_tensor(out=c_t[d][:], in0=t1[:], in1=t2[:], op=OP.add)
                            else:
                                nc.vector.tensor_copy(out=c_t[d][:], in_=t1[:])
                            tc_ = sp.tile([8, H], f32, name=f"tc{d}", tag=f"tc{d}")
                            nc.scalar.activation(out=tc_[:], in_=c_t[d][:], func=AF.Tanh)
                            hh = sp.tile([8, H], f32, name=f"hh{d}", tag=f"hh{d}")
                            nc.vector.tensor_tensor(out=hh[:], in0=sig[:, 512:768], in1=tc_[:], op=OP.mult)
                            # transpose h -> hT columns
                            tp = tpp.tile([128, 16], f32, name=f"tp{d}", tag=f"tp{d}", space="PSUM")
                            col = tok[d] * 8
                            base = 0 if d == "f" else 2
                            for k in range(2):
                                nc.tensor.transpose(out=tp[:, k * 8:(k + 1) * 8],
                                                    in_=hh[:, k * 128:(k + 1) * 128],
                                                    identity=ident[0:8, 0:8])
                            nc.scalar.copy(out=hT[base][:, col:col + 8], in_=tp[:, 0:8])
                            nc.vector.tensor_copy(out=hT[base + 1][:, col:col + 8], in_=tp[:, 8:16])

            lstm_layer(0, h0T)

            # ---------------- bulk X1 = h0 @ w_ih1T + b1 ----------------
            with tc.tile_pool(name="bw", bufs=1) as bwp, \
                 tc.tile_pool(name="bs", bufs=3) as bsp, \
                 tc.tile_pool(name="bp", bufs=2, space="PSUM") as bpp:
                wih1 = {}
                for d in "fb":
                    wih1[d] = [bwp.tile([128, G], f32, name=f"wih1{d}{k}", tag=f"wih1{d}{k}") for k in range(4)]
                    for k in range(4):
                        nc.sync.dma_start(wih1[d][k][:], wih1_d[d][k * 128:(k + 1) * 128, :])
                for d in "fb":
                    mt_order = list(range(NT // 128))
                    if d == "b":
                        mt_order = mt_order[::-1]
                    for m in mt_order:
                        px = bpp.tile([128, G], f32, name="px", tag="px", space="PSUM")
                        for hf in range(2):
                            colsl = slice(hf * 512, (hf + 1) * 512)
                            nc.tensor.matmul(px[:, colsl], lhsT=ones[:, 0:128],
                                             rhs=b1[d][:, colsl], start=True, stop=False)
                            for k in range(4):
                                nc.tensor.matmul(px[:, colsl], lhsT=h0T[k][:, m * 128:(m + 1) * 128],
                                                 rhs=wih1[d][k][:, colsl], start=False, stop=(k == 3))
                        st = bsp.tile([128, G], f32, name="st", tag="st")
                        nc.scalar.copy(out=st[:], in_=px[:])
                        nc.sync.dma_start(X1dram[d][m * 128:(m + 1) * 128, :], st[:])

            ph0_cm.__exit__(None, None, None)
            ph1_cm = tc.tile_pool(name="ph1", bufs=1)
            ph1 = ph1_cm.__enter__()
            h1T = [ph1.tile([128, NT], f32, name=f"h1T{i}", tag=f"h1T{i}") for i in range(4)]
            lstm_layer(1, h1T)

            # ---------------- tag scores ----------------
            tags = pp.tile([128, CC * T], f32, name="tags", tag="tags")
            with tc.tile_pool(name="cw", bufs=1) as cwp, \
                 tc.tile_pool(name="cp", bufs=2, space="PSUM") as cpp:
                clsT = [cwp.tile([128, T], f32, name=f"clsT{k}", tag=f"clsT{k}") for k in range(4)]
                for k in range(4):
                    nc.sync.dma_start(clsT[k][:], cls_d[k * 128:(k + 1) * 128, :])
                clsb = cwp.tile([1, T], f32, name="clsb", tag="clsb")
                nc.sync.dma_start(clsb[:], clsb_d[:])
                for m in range(CC):
                    pt = cpp.tile([128, T], f32, name="pt", tag="pt", space="PSUM")
                    nc.tensor.matmul(pt[:], lhsT=ones[:, 0:128], rhs=clsb[:], start=True, stop=False)
                    for k in range(4):
                        nc.tensor.matmul(pt[:], lhsT=h1T[k][:, m * 128:(m + 1) * 128],
                                         rhs=clsT[k][:], start=False, stop=(k == 3))
                    nc.scalar.copy(out=tags[:, m * T:(m + 1) * T], in_=pt[:])
            nc.sync.dma_start(preds_d[:], tags[:])
            ph1_cm.__exit__(None, None, None)

            # ---------------- CRF ----------------
            with tc.tile_pool(name="crf", bufs=1) as cf, \
                 tc.tile_pool(name="crfp", bufs=2, space="PSUM") as cfp:
                # replicate constants across partitions via ones-matmul
                def rep(dram, n):
                    ps = cfp.tile([128, n], f32, name="rep", tag="rep", space="PSUM")
                    sb = cf.tile([128, n], f32, name=f"rep{dram.name}", tag=f"rep{dram.name}")
                    nc.tensor.matmul(ps[:], lhsT=ones[:], rhs=dram_sb[dram.name][:], start=True, stop=True)
                    nc.vector.tensor_copy(out=sb[:], in_=ps[:])
                    return sb
                dram_sb = {}
                for dd, n in ((start_d, T), (end_d, T), (trm_d, T * T), (trj_d, T * T)):
                    s = cf.tile([1, n], f32, name=f"ld{dd.name}", tag=f"ld{dd.name}")
                    nc.sync.dma_start(s[:], dd[:])
                    dram_sb[dd.name] = s
                startR = rep(start_d, T)
                endR = rep(end_d, T)
                trmR = rep(trm_d, T * T)
                trjR = rep(trj_d, T * T)
                labs = cf.tile([128, CC], i32, name="labs", tag="labs")
                nc.sync.dma_start(labs[:], lab_d[:])
                labsp = cf.tile([128, CC], i32, name="labsp", tag="labsp")
                nc.sync.dma_start(labsp[:], labp_d[:])
                maskf = cf.tile([128, CC], f32, name="maskf", tag="maskf")
                nc.sync.dma_start(maskf[:], mask_d[:])
                masknx = cf.tile([128, CC], f32, name="masknx", tag="masknx")
                nc.sync.dma_start(masknx[:], masknx_d[:])

                labf = cf.tile([128, CC], f32, name="labf", tag="labf")
                nc.vector.tensor_copy(out=labf[:], in_=labs[:])
                labpf = cf.tile([128, CC], f32, name="labpf", tag="labpf")
                nc.vector.tensor_copy(out=labpf[:], in_=labsp[:])

                io9 = cf.tile([128, T], i32, name="io9", tag="io9")
                nc.gpsimd.iota(io9[:], pattern=[[1, T]], base=0, channel_multiplier=0)
                io9f = cf.tile([128, T], f32, name="io9f", tag="io9f")
                nc.vector.tensor_copy(out=io9f[:], in_=io9[:])
                io81 = cf.tile([128, T * T], i32, name="io81", tag="io81")
                nc.gpsimd.iota(io81[:], pattern=[[1, T * T]], base=0, channel_multiplier=0)
                io81f = cf.tile([128, T * T], f32, name="io81f", tag="io81f")
                nc.vector.tensor_copy(out=io81f[:], in_=io81[:])

                # one-hot(labels) [128, CC*T]
                oh = cf.tile([128, CC * T], f32, name="oh", tag="oh")
                nc.vector.tensor_tensor(out=oh[:], in0=labf[:].rearrange("p (c o) -> p c o", o=1).to_broadcast([128, CC, T]),
                                        in1=io9f[:].rearrange("p (o t) -> p o t", o=1).to_broadcast([128, CC, T]),
                                        op=OP.is_equal)
                emsel = cf.tile([128, CC], f32, name="emsel", tag="emsel")
                tmp = cf.tile([128, CC * T], f32, name="tmpA", tag="tmpA")
                nc.vector.tensor_tensor(out=tmp[:], in0=oh[:], in1=tags[:], op=OP.mult)
                nc.vector.tensor_reduce(out=emsel[:], in_=tmp[:].rearrange("p (c t) -> p c t", t=T),
                                        axis=AX.X, op=OP.add)
                endsel = cf.tile([128, CC], f32, name="endsel", tag="endsel")
                nc.vector.tensor_tensor(out=tmp[:], in0=oh[:],
                                        in1=endR[:].rearrange("p (o t) -> p o t", o=1).to_broadcast([128, CC, T]), op=OP.mult)
                nc.vector.tensor_reduce(out=endsel[:], in_=tmp[:].rearrange("p (c t) -> p c t", t=T),
                                        axis=AX.X, op=OP.add)
                # trans[prev, cur]
                pairf = cf.tile([128, CC], f32, name="pairf", tag="pairf")
                nc.vector.scalar_tensor_tensor(out=pairf[:], in0=labpf[:], scalar=float(T), in1=labf[:],
                                               op0=OP.mult, op1=OP.add)
                oh81 = cf.tile([128, CC * T * T], f32, name="oh81", tag="oh81")
                nc.vector.tensor_tensor(out=oh81[:],
                                        in0=pairf[:].rearrange("p (c o) -> p c o", o=1).to_broadcast([128, CC, T * T]),
                                        in1=io81f[:].rearrange("p (o t) -> p o t", o=1).to_broadcast([128, CC, T * T]),
                                        op=OP.is_equal)
                tmp81 = cf.tile([128, CC * T * T], f32, name="tmp81", tag="tmp81")
                nc.vector.tensor_tensor(out=tmp81[:], in0=oh81[:],
                                        in1=trmR[:].rearrange("p (o t) -> p o t", o=1).to_broadcast([128, CC, T * T]),
                                        op=OP.mult)
                trsel = cf.tile([128, CC], f32, name="trsel", tag="trsel")
                nc.vector.tensor_reduce(out=trsel[:], in_=tmp81[:].rearrange("p (c t) -> p c t", t=T * T),
                                        axis=AX.X, op=OP.add)
                # masks
                m0 = cf.tile([128, CC], f32, name="m0", tag="m0")
                nc.vector.tensor_copy(out=m0[:], in_=maskf[:])
                nc.gpsimd.memset(m0[0:Bc, 0:1], 1.0)
                mp = cf.tile([128, CC], f32, name="mp", tag="mp")
                nc.vector.tensor_copy(out=mp[:], in_=maskf[:])
                nc.gpsimd.memset(mp[0:Bc, 0:1], 0.0)
                ind = cf.tile([128, CC], f32, name="ind", tag="ind")
                nc.vector.tensor_tensor(out=ind[:], in0=maskf[:], in1=masknx[:], op=OP.subtract)
                # val = m0*emsel + mp*trsel + ind*endsel  (+ start_sel at t=0)
                val = cf.tile([128, CC], f32, name="val", tag="val")
                nc.vector.tensor_tensor(out=val[:], in0=m0[:], in1=emsel[:], op=OP.mult)
                v2 = cf.tile([128, CC], f32, name="v2", tag="v2")
                nc.vector.tensor_tensor(out=v2[:], in0=mp[:], in1=trsel[:], op=OP.mult)
                nc.vector.tensor_tensor(out=val[:], in0=val[:], in1=v2[:], op=OP.add)
                nc.vector.tensor_tensor(out=v2[:], in0=ind[:], in1=endsel[:], op=OP.mult)
                nc.vector.tensor_tensor(out=val[:], in0=val[:], in1=v2[:], op=OP.add)
                ssel = cf.tile([Bc, 1], f32, name="ssel", tag="ssel")
                stmp = cf.tile([Bc, T], f32, name="stmp", tag="stmp")
                nc.vector.tensor_tensor(out=stmp[:], in0=oh[0:Bc, 0:T], in1=startR[0:Bc, :], op=OP.mult)
                nc.vector.tensor_reduce(out=ssel[:], in_=stmp[:], axis=AX.X, op=OP.add)
                nc.vector.tensor_tensor(out=val[0:Bc, 0:1], in0=val[0:Bc, 0:1], in1=ssel[:], op=OP.add)
                # bsel[p, b] = (p % 8 == b), host-provided constant
                bsel = cf.tile([128, Bc], f32, name="bsel", tag="bsel")
                nc.sync.dma_start(bsel[:], bsel_d[:])
                nps = cfp.tile([Bc, CC], f32, name="nps", tag="nps", space="PSUM")
                nc.tensor.matmul(nps[:], lhsT=bsel[:], rhs=val[:], start=True, stop=True)
                num = cf.tile([Bc, 1], f32, name="num", tag="num")
                nc.vector.tensor_reduce(out=num[:], in_=nps[:], axis=AX.X, op=OP.add)

                # ---- normalizer: sequential scan ----
                Ptab = cf.tile([Bc, T * T], f32, name="Ptab", tag="Ptab")   # exp(trans) j-major
                nc.scalar.activation(out=Ptab[:], in_=trjR[0:Bc, :], func=AF.Exp)
                score = cf.tile([Bc, T], f32, name="score", tag="score")
                noff = cf.tile([Bc, 1], f32, name="noff", tag="noff")
                emp = cfp.tile([Bc, T], f32, name="emp", tag="emp", space="PSUM")
                nc.tensor.matmul(emp[:], lhsT=ident[:, 0:Bc], rhs=tags[:, 0:T], start=True, stop=True)
                nc.vector.tensor_tensor(out=score[:], in0=startR[0:Bc, :], in1=emp[:], op=OP.add)
                nc.vector.tensor_scalar(out=noff[:], in0=score[:, 0:1], scalar1=-1.0, scalar2=None, op0=OP.mult)
                pexp = cf.tile([Bc, T], f32, name="pexp", tag="pexp")
                qt = cf.tile([Bc, T * T], f32, name="qt", tag="qt")
                qs = cf.tile([Bc, T], f32, name="qs", tag="qs")
                lnq = cf.tile([Bc, T], f32, name="lnq", tag="lnq")
                for t in range(1, Lb):
                    emp2 = cfp.tile([Bc, T], f32, name="emp", tag="emp", space="PSUM")
                    r = t * Bc
                    nc.tensor.matmul(emp2[:], lhsT=ident[:, (r % 128):(r % 128) + Bc],
                                     rhs=tags[:, (r // 128) * T:(r // 128) * T + T], start=True, stop=True)
                    nc.scalar.activation(out=pexp[:], in_=score[:], func=AF.Exp, bias=noff[:, 0:1])
                    nc.vector.tensor_tensor(out=qt[:],
                                            in0=pexp[:].rearrange("p (o t) -> p o t", o=1).to_broadcast([Bc, T, T]),
                                            in1=Ptab[:].rearrange("p (j i) -> p j i", i=T),
                                            op=OP.mult)
                    nc.vector.tensor_reduce(out=qs[:], in_=qt[:].rearrange("p (j i) -> p j i", i=T),
                                            axis=AX.X, op=OP.add)
                    nc.scalar.activation(out=lnq[:], in_=qs[:], func=AF.Ln)
                    nc.vector.scalar_tensor_tensor(out=score[:], in0=lnq[:], scalar=noff[:, 0:1],
                                                   in1=emp2[:], op0=OP.subtract, op1=OP.add)
                    nc.vector.tensor_scalar(out=noff[:], in0=score[:, 0:1], scalar1=-1.0, scalar2=None, op0=OP.mult)
                # Z = LSE(score + end)
                fin = cf.tile([Bc, T], f32, name="fin", tag="fin")
                nc.vector.tensor_tensor(out=fin[:], in0=score[:], in1=endR[0:Bc, :], op=OP.add)
                mx = cf.tile([Bc, 1], f32, name="mx", tag="mx")
                nc.vector.tensor_reduce(out=mx[:], in_=fin[:], axis=AX.X, op=OP.max)
                nmx = cf.tile([Bc, 1], f32, name="nmx", tag="nmx")
                nc.vector.tensor_scalar(out=nmx[:], in0=mx[:], scalar1=-1.0, scalar2=None, op0=OP.mult)
                nc.scalar.activation(out=fin[:], in_=fin[:], func=AF.Exp, bias=nmx[:, 0:1])
                sm = cf.tile([Bc, 1], f32, name="sm", tag="sm")
                nc.vector.tensor_reduce(out=sm[:], in_=fin[:], axis=AX.X, op=OP.add)
                lsm = cf.tile([Bc, 1], f32, name="lsm", tag="lsm")
                nc.scalar.activation(out=lsm[:], in_=sm[:], func=AF.Ln)
                zz = cf.tile([Bc, 1], f32, name="zz", tag="zz")
                nc.vector.tensor_tensor(out=zz[:], in0=lsm[:], in1=mx[:], op=OP.add)
                llh = cf.tile([Bc, 1], f32, name="llhv", tag="llhv")
                nc.vector.tensor_tensor(out=llh[:], in0=num[:], in1=zz[:], op=OP.subtract)
                nc.sync.dma_start(llh_d[:], llh[:])

    nc.compile()
    return nc


def _prep_inputs(Lb, input_ids, labels, attention_mask, emb,
                 w_ih_l0f, w_hh_l0f, b_l0f, w_ih_l0b, w_hh_l0b, b_l0b,
                 w_ih_l1f, w_hh_l1f, b_l1f, w_ih_l1b, w_hh_l1b, b_l1b,
                 cls_w, cls_b, start_trans, end_trans, trans):
    f = np.float32
    NT = Lb * Bc
    NC2 = NT // 128
    CC = NT // 128
    com = {
        "emb": np.ascontiguousarray(emb, f),
        "ident": np.eye(128, dtype=f),
        "ones": np.ones((1, 128), f),
        "clsT": np.ascontiguousarray(cls_w.T, f),
        "clsb": np.ascontiguousarray(cls_b.reshape(1, T), f),
        "startv": np.ascontiguousarray(start_trans.reshape(1, T), f),
        "endv": np.ascontiguousarray(end_trans.reshape(1, T), f),
        "trans_rm": np.ascontiguousarray(trans.reshape(1, T * T), f),
        "trans_jm": np.ascontiguousarray(trans.T.reshape(1, T * T), f),
        "bsel": np.fromfunction(lambda p, b: (p % 8 == b).astype(np.float32), (128, 8)).astype(f),
    }
    for d, wi, wh, bb in (("f", w_ih_l0f, w_hh_l0f, b_l0f), ("b", w_ih_l0b, w_hh_l0b, b_l0b)):
        com[f"wih0{d}"] = np.ascontiguousarray(wi[PERM].T, f)
        com[f"whh0{d}"] = np.ascontiguousarray(wh[PERM].T, f)
        com[f"b0{d}"] = np.ascontiguousarray(bb[PERM].reshape(1, G), f)
    for d, wi, wh, bb in (("f", w_ih_l1f, w_hh_l1f, b_l1f), ("b", w_ih_l1b, w_hh_l1b, b_l1b)):
        com[f"wih1{d}"] = np.ascontiguousarray(wi[PERM].T, f)
        com[f"whh1{d}"] = np.ascontiguousarray(wh[PERM].T, f)
        com[f"b1{d}"] = np.ascontiguousarray(bb[PERM].reshape(1, G), f)

    in_maps = []
    for c in range(NCORES):
        ids = input_ids[c * Bc:(c + 1) * Bc, :Lb]          # [8, Lb]
        lab = labels[c * Bc:(c + 1) * Bc, :Lb]
        msk = attention_mask[c * Bc:(c + 1) * Bc, :Lb]
        idx2 = np.ascontiguousarray(ids.T).reshape(-1)     # tok = t*8+b
        m = dict(com)
        m["idx2"] = np.ascontiguousarray(idx2.reshape(NC2, 128).T.astype(np.int32))
        labt = np.ascontiguousarray(lab.T).reshape(-1)     # tok = t*8+b
        m["labs"] = np.ascontiguousarray(labt.reshape(CC, 128).T.astype(np.int32))
        labp = np.roll(labt, Bc)
        m["labsp"] = np.ascontiguousarray(labp.reshape(CC, 128).T.astype(np.int32))
        mskt = np.ascontiguousarray(msk.T).reshape(-1).astype(f)
        m["maskf"] = np.ascontiguousarray(mskt.reshape(CC, 128).T)
        msknx = np.concatenate([mskt[Bc:], np.zeros(Bc, f)])
        m["masknx"] = np.ascontiguousarray(msknx.reshape(CC, 128).T)
        in_maps.append(m)
    return in_maps


def run(Lb, inputs, trace=False):
    from concourse import bass_utils
    if Lb not in _CACHE:
        _CACHE[Lb] = _build(Lb)
    nc = _CACHE[Lb]
    in_maps = _prep_inputs(Lb, **inputs)
    res = bass_utils.run_bass_kernel_spmd(nc, in_maps, core_ids=list(range(NCORES)), trace=trace)
    NT = Lb * Bc
    CC = NT // 128
    preds = np.empty((B, Lb, T), np.float32)
    loss = np.float32(0.0)
    for c in range(NCORES):
        raw = res.results[c]["preds_raw"].reshape(128, CC, T)
        y = raw.transpose(1, 0, 2).reshape(NT, T)          # tok = t*8+b
        preds[c * Bc:(c + 1) * Bc, :, :] = y.reshape(Lb, Bc, T).transpose(1, 0, 2)
        loss -= res.results[c]["llh"].sum()
    return (preds, np.float32(loss)), res


def kernel(**inputs):
    (preds, loss), _ = run(L, inputs)
    return preds, loss
